# revision 4
# baseline (speedup 1.0000x reference)
"""Bass/Trainium2 kernel for nn_ExaoneMoEAttention (sliding-window GQA attention).

Strategy (8 NeuronCores, tensor-parallel over heads):
  - core c owns q heads 4c..4c+3 and kv head c (w_qkv column shard [4096, 768]),
    plus w_o rows 512c..512c+512 ([512, 4096]).
  - hidden is replicated (passed host-transposed as hidT [4096, 2048]).
  - QKV projection computed in [dim, t] layout (dim on partitions) with fp32r
    matmuls; per-head RMSNorm uses a ones-column matmul for the partition-dim
    reduction, RoPE uses host-precomputed cos/sin tables (halves duplicated).
  - Attention: scoresT[s, t] tiles of [128, 512]; sliding window (1024) +
    causal handled block-sparsely (<=12 key tiles per 512-wide q chunk) with
    multiplicative 0/1 masks; softmax without max-subtraction (RMSNorm bounds
    |score| <= sqrt(D)); exp-sum via ones-matmul; unnormalized attn @ v
    accumulated in PSUM; normalization by broadcast reciprocal.
  - o_proj per 512-row slab, then ReduceScatter(add) over the 8 cores per
    slab (overlaps with later slabs); host concatenates the 8 row-shards.
"""

import numpy as np

import concourse.bass as bass
import concourse.mybir as mybir
import concourse.tile as tile
from concourse import bacc
from concourse.bass_utils import run_bass_kernel_spmd
from concourse.masks import make_identity

F32 = mybir.dt.float32
F32R = mybir.dt.float32r
AF = mybir.ActivationFunctionType

N_CORES = 8
T = 2048
HID = 4096
H = 32
HKV = 8
D = 128
EPS = 1e-5
THETA = 1e6
WINDOW = 1024

HL = H // N_CORES          # 4 local q heads
CHUNK_A = 256              # t-chunk in QKV projection phase
N_CHUNKS_A = T // CHUNK_A  # 8
QC = 512                   # q chunk in attention phase
N_QC = T // QC             # 4
N_ST = T // 128            # 16 s-tiles
KO = HID // 128            # 32 k-subtiles in projection
ECH = 512                  # o_proj e-chunk
N_ECH = HID // ECH         # 8

# deltas (t_chunk_start - s_tile_start) that need masks, in host mask order
MASK_DELTAS = [0, -128, -256, -384, 640, 768, 896, 1024]
MASK_IDX = {d: i for i, d in enumerate(MASK_DELTAS)}


def _build():
    nc = bacc.Bacc(num_devices=N_CORES)

    hidT = nc.declare_dram_parameter("hidT", [HID, T], F32R, isOutput=False)
    wq = nc.declare_dram_parameter("wq", [HID, (HL + 2) * D], F32R, isOutput=False)
    wo = nc.declare_dram_parameter("wo", [HL * D, HID], F32R, isOutput=False)
    cs2 = nc.declare_dram_parameter("cs2", [128, T], F32, isOutput=False)
    sn2s = nc.declare_dram_parameter("sn2s", [128, T], F32, isOutput=False)
    masks = nc.declare_dram_parameter("masks", [len(MASK_DELTAS), 128, QC], F32, isOutput=False)
    qwv = nc.declare_dram_parameter("qwv", [D, 1], F32, isOutput=False)
    kwv = nc.declare_dram_parameter("kwv", [D, 1], F32, isOutput=False)
    onc_d = nc.declare_dram_parameter("onc", [128, 1], F32R, isOutput=False)
    onr_d = nc.declare_dram_parameter("onr", [1, 128], F32R, isOutput=False)
    out_p = nc.declare_dram_parameter("out", [N_QC, QC // N_CORES, HID], F32, isOutput=True)

    with tile.TileContext(nc) as tc:
        with (
            tc.tile_pool(name="persistA", bufs=1) as pA,
            tc.tile_pool(name="dramA", bufs=1, space="DRAM") as dramA,
        ):
            kT = pA.tile([128, T], F32R)           # rope'd k, [d, s]
            vnat = pA.tile([128, N_ST, D], F32R)   # v in [s, d] tiles
            onc = pA.tile([128, 1], F32R)
            onr = pA.tile([1, 128], F32R)
            nc.sync.dma_start(out=onc[:], in_=onc_d[:])
            nc.sync.dma_start(out=onr[:], in_=onr_d[:])

            qT_dram = [dramA.tile([128, T], F32R, name=f"qT{h}") for h in range(HL)]

            # ---------------- Phase A: QKV projection + norm + rope ----------
            with (
                tc.tile_pool(name="wpool", bufs=1) as wpool,
                tc.tile_pool(name="hidp", bufs=2) as hidp,
                tc.tile_pool(name="cspool", bufs=2) as cspool,
                tc.tile_pool(name="tmpA", bufs=2) as tmpA,
                tc.tile_pool(name="stA", bufs=2) as stA,
                tc.tile_pool(name="miscA", bufs=1) as miscA,
                tc.tile_pool(name="psq", bufs=2, space="PSUM") as psq_p,
                tc.tile_pool(name="psst", bufs=2, space="PSUM") as psst_p,
                tc.tile_pool(name="psbc", bufs=2, space="PSUM") as psbc_p,
                tc.tile_pool(name="psvt", bufs=2, space="PSUM") as psvt_p,
            ):
                w_sb = wpool.tile([128, KO, (HL + 2) * D], F32R)
                nc.sync.dma_start(
                    out=w_sb[:], in_=wq.rearrange("(ko ki) m -> ki ko m", ki=128)
                )
                qw_sb = miscA.tile([D, 1], F32)
                kw_sb = miscA.tile([D, 1], F32)
                ident = miscA.tile([128, 128], F32)
                eps_sb = miscA.tile([1, 1], F32)
                nc.sync.dma_start(out=qw_sb[:], in_=qwv[:])
                nc.sync.dma_start(out=kw_sb[:], in_=kwv[:])
                make_identity(nc, ident[:])
                nc.vector.memset(eps_sb[:], EPS)

                hidT_r = hidT.rearrange("(ko ki) t -> ki ko t", ki=128)

                for tci in range(N_CHUNKS_A):
                    tsl = slice(tci * CHUNK_A, (tci + 1) * CHUNK_A)
                    hid_t = hidp.tile([128, KO, CHUNK_A], F32R, tag="hid")
                    nc.sync.dma_start(out=hid_t[:], in_=hidT_r[:, :, tsl])
                    cs_t = cspool.tile([128, CHUNK_A], F32, tag="cs")
                    sn_t = cspool.tile([128, CHUNK_A], F32, tag="sn")
                    nc.sync.dma_start(out=cs_t[:], in_=cs2[:, tsl])
                    nc.sync.dma_start(out=sn_t[:], in_=sn2s[:, tsl])

                    for m in range(HL + 2):
                        psq = psq_p.tile([128, CHUNK_A], F32, tag="psq")
                        for ko in range(KO):
                            nc.tensor.matmul(
                                psq[:],
                                w_sb[:, ko, m * D:(m + 1) * D],
                                hid_t[:, ko, :],
                                start=(ko == 0),
                                stop=(ko == KO - 1),
                            )
                        if m < HL + 1:
                            # --- RMSNorm (partition-dim reduce via matmul) ---
                            sq = tmpA.tile([128, CHUNK_A], F32R, tag="sq")
                            nc.scalar.activation(sq[:], psq[:], AF.Square)
                            ssum = psst_p.tile([1, CHUNK_A], F32, tag="ssum")
                            nc.tensor.matmul(ssum[:], onc[:], sq[:], start=True, stop=True)
                            vtmp = stA.tile([1, CHUNK_A], F32, tag="vtmp")
                            nc.scalar.activation(
                                vtmp[:], ssum[:], AF.Sqrt, scale=1.0 / D, bias=eps_sb[:]
                            )
                            rstd = stA.tile([1, CHUNK_A], F32R, tag="rstd")
                            with nc.allow_low_precision(reason="f32r rstd for broadcast matmul"):
                                nc.vector.reciprocal(rstd[:], vtmp[:])
                            bcp = psbc_p.tile([128, CHUNK_A], F32, tag="bcp")
                            nc.tensor.matmul(bcp[:], onr[:], rstd[:], start=True, stop=True)
                            # apply norm weight (and softmax scale for q) + rstd
                            qn = tmpA.tile([128, CHUNK_A], F32, tag="qn")
                            nc.scalar.activation(
                                qn[:], psq[:], AF.Copy,
                                scale=(qw_sb[:] if m < HL else kw_sb[:]),
                            )
                            nc.vector.tensor_mul(qn[:], qn[:], bcp[:])
                            # --- RoPE (neox rotate-halves) ---
                            qsw = tmpA.tile([128, CHUNK_A], F32, tag="qsw")
                            nc.vector.tensor_copy(qsw[0:64, :], qn[64:128, :])
                            nc.vector.tensor_copy(qsw[64:128, :], qn[0:64, :])
                            nc.vector.tensor_mul(qn[:], qn[:], cs_t[:])
                            nc.vector.tensor_mul(qsw[:], qsw[:], sn_t[:])
                            if m < HL:
                                stg = tmpA.tile([128, CHUNK_A], F32R, tag="stg")
                                nc.vector.tensor_add(stg[:], qn[:], qsw[:])
                                nc.sync.dma_start(out=qT_dram[m][:, tsl], in_=stg[:])
                            else:
                                nc.vector.tensor_add(kT[:, tsl], qn[:], qsw[:])
                        else:
                            # --- v: copy + transpose into [s, d] tiles ---
                            vstg = tmpA.tile([128, CHUNK_A], F32, tag="vstg")
                            nc.scalar.activation(vstg[:], psq[:], AF.Copy)
                            for j in range(CHUNK_A // 128):
                                vt_ps = psvt_p.tile([128, 128], F32, tag="vt")
                                nc.tensor.transpose(
                                    vt_ps[:], vstg[:, j * 128:(j + 1) * 128], ident[:]
                                )
                                st_i = tci * (CHUNK_A // 128) + j
                                nc.vector.tensor_copy(vnat[:, st_i, :], vt_ps[:])

            # ---------------- Phase B: attention + o_proj + reduce-scatter ---
            with (
                tc.tile_pool(name="persistB", bufs=1) as pB,
                tc.tile_pool(name="qvp", bufs=3) as qvp,
                tc.tile_pool(name="exp", bufs=4) as exp_p,
                tc.tile_pool(name="stB", bufs=2) as stB,
                tc.tile_pool(name="ostg", bufs=3) as ostg_p,
                tc.tile_pool(name="psA", bufs=2, space="PSUM") as psA_p,
                tc.tile_pool(name="psav", bufs=2, space="PSUM") as psav_p,
                tc.tile_pool(name="pssum", bufs=2, space="PSUM") as pssum_p,
                tc.tile_pool(name="psbcB", bufs=1, space="PSUM") as psbcB_p,
                tc.tile_pool(name="dramB", bufs=1, space="DRAM") as dramB,
            ):
                attnT = pB.tile([128, HL, T], F32R)
                wo_sb = pB.tile([128, HL, HID], F32R)
                mask_sb = pB.tile([128, len(MASK_DELTAS), QC], F32)
                nc.sync.dma_start(
                    out=wo_sb[:], in_=wo.rearrange("(h p) e -> p h e", p=128)
                )
                nc.sync.dma_start(
                    out=mask_sb[:], in_=masks.rearrange("m p f -> p m f")
                )

                partial = [
                    dramB.tile([QC, HID], F32, name=f"partial{qc}") for qc in range(N_QC)
                ]
                rs_out = [
                    dramB.tile([QC // N_CORES, HID], F32, name=f"rsout{qc}")
                    for qc in range(N_QC)
                ]

                for qc in range(N_QC):
                    qsl = slice(qc * QC, (qc + 1) * QC)
                    si_lo = max(0, 4 * qc - 8)
                    si_hi = 4 * qc + 3
                    sis = list(range(si_lo, si_hi + 1))
                    for h in range(HL):
                        qv = qvp.tile([128, QC], F32R, tag="qv")
                        nc.sync.dma_start(out=qv[:], in_=qT_dram[h][:, qsl])
                        ps_av = psav_p.tile([128, QC], F32, tag="av")
                        ps_sum = pssum_p.tile([1, QC], F32, tag="sum")
                        exs = {}
                        # software-pipelined: scores run one s-tile ahead of
                        # the exp-consuming matmuls
                        def emit_scores(si):
                            psc = psA_p.tile([128, QC], F32, tag="sc")
                            nc.tensor.matmul(
                                psc[:], kT[:, si * 128:(si + 1) * 128], qv[:],
                                start=True, stop=True,
                            )
                            ex = exp_p.tile([128, QC], F32R, tag="ex")
                            nc.scalar.activation(ex[:], psc[:], AF.Exp)
                            delta = qc * QC - si * 128
                            if delta in MASK_IDX:
                                nc.vector.tensor_mul(
                                    ex[:], ex[:], mask_sb[:, MASK_IDX[delta], :]
                                )
                            exs[si] = ex

                        def emit_consume(si):
                            ex = exs.pop(si)
                            first = si == sis[0]
                            last = si == sis[-1]
                            nc.tensor.matmul(
                                ps_sum[:], onc[:], ex[:], start=first, stop=last
                            )
                            nc.tensor.matmul(
                                ps_av[:], vnat[:, si, :], ex[:], start=first, stop=last
                            )

                        emit_scores(sis[0])
                        for si in sis[1:]:
                            emit_scores(si)
                            emit_consume(si - 1)
                        emit_consume(sis[-1])

                        rc = stB.tile([1, QC], F32R, tag="rc")
                        with nc.allow_low_precision(reason="f32r recip for broadcast matmul"):
                            nc.vector.reciprocal(rc[:], ps_sum[:])
                        bcp = psbcB_p.tile([128, QC], F32, tag="bcB")
                        nc.tensor.matmul(bcp[:], onr[:], rc[:], start=True, stop=True)
                        at = attnT[:, h, qsl]
                        nc.scalar.activation(at, ps_av[:], AF.Copy)
                        nc.vector.tensor_mul(at, at, bcp[:])

                    # o_proj for this 512-row slab
                    for tt in range(QC // 128):
                        trow = qc * 4 + tt
                        for ec in range(N_ECH):
                            pso = psA_p.tile([128, ECH], F32, tag="sc")
                            for h in range(HL):
                                nc.tensor.matmul(
                                    pso[:],
                                    attnT[:, h, trow * 128:(trow + 1) * 128],
                                    wo_sb[:, h, ec * ECH:(ec + 1) * ECH],
                                    start=(h == 0),
                                    stop=(h == HL - 1),
                                )
                            ost = ostg_p.tile([128, ECH], F32, tag="ost")
                            nc.any.tensor_copy(out=ost[:], in_=pso[:])
                            nc.sync.dma_start(
                                out=partial[qc][tt * 128:(tt + 1) * 128,
                                                ec * ECH:(ec + 1) * ECH],
                                in_=ost[:],
                            )
                    nc.gpsimd.collective_compute(
                        "ReduceScatter",
                        mybir.AluOpType.add,
                        replica_groups=[list(range(N_CORES))],
                        ins=[partial[qc][:]],
                        outs=[rs_out[qc][:]],
                    )
                    nc.sync.dma_start(out=out_p[qc], in_=rs_out[qc][:])

    nc.finalize()
    return nc


_NC_CACHE = None


def _get_nc():
    global _NC_CACHE
    if _NC_CACHE is None:
        _NC_CACHE = _build()
    return _NC_CACHE


def _host_inputs(positions, hidden_states, w_qkv, q_norm_w, k_norm_w, w_o):
    positions = np.asarray(positions)
    hidden_states = np.asarray(hidden_states, dtype=np.float32)
    w_qkv = np.asarray(w_qkv, dtype=np.float32)
    q_norm_w = np.asarray(q_norm_w, dtype=np.float32)
    k_norm_w = np.asarray(k_norm_w, dtype=np.float32)
    w_o = np.asarray(w_o, dtype=np.float32)

    hidT = np.ascontiguousarray(hidden_states.T)

    half = D // 2
    inv_freq = 1.0 / (THETA ** (np.arange(half, dtype=np.float32) / half))
    ang = positions.astype(np.float32)[:, None] * inv_freq[None, :]  # [T, 64]
    cos = np.cos(ang).T.astype(np.float32)   # [64, T]
    sin = np.sin(ang).T.astype(np.float32)
    cs2 = np.concatenate([cos, cos], axis=0)          # [128, T]
    sn2s = np.concatenate([-sin, sin], axis=0)        # [128, T]

    mk = np.zeros((len(MASK_DELTAS), 128, QC), np.float32)
    ss = np.arange(128)[:, None]
    ttv = np.arange(QC)[None, :]
    for i, dlt in enumerate(MASK_DELTAS):
        diff = dlt + ttv - ss
        mk[i] = ((diff >= 0) & (diff < WINDOW)).astype(np.float32)

    qwv = (q_norm_w * (D ** -0.5)).reshape(D, 1).astype(np.float32)
    kwv = k_norm_w.reshape(D, 1).astype(np.float32)
    onc = np.ones((128, 1), np.float32)
    onr = np.ones((1, 128), np.float32)

    in_maps = []
    for c in range(N_CORES):
        wq_c = np.ascontiguousarray(
            np.concatenate(
                [
                    w_qkv[:, c * HL * D:(c + 1) * HL * D],
                    w_qkv[:, H * D + c * D:H * D + (c + 1) * D],
                    w_qkv[:, (H + HKV) * D + c * D:(H + HKV) * D + (c + 1) * D],
                ],
                axis=1,
            )
        )
        wo_c = np.ascontiguousarray(w_o[c * HL * D:(c + 1) * HL * D, :])
        in_maps.append(
            {
                "hidT": hidT,
                "wq": wq_c,
                "wo": wo_c,
                "cs2": cs2,
                "sn2s": sn2s,
                "masks": mk,
                "qwv": qwv,
                "kwv": kwv,
                "onc": onc,
                "onr": onr,
            }
        )
    return in_maps


def _assemble(results):
    out = np.empty((T, HID), np.float32)
    rows = QC // N_CORES
    for c in range(N_CORES):
        r = results[c]["out"]  # [N_QC, rows, HID]
        for qc in range(N_QC):
            out[qc * QC + c * rows: qc * QC + (c + 1) * rows] = r[qc]
    return out


def run_spmd(in_maps, trace=False, **kw):
    nc = _get_nc()
    return run_bass_kernel_spmd(nc, in_maps, list(range(N_CORES)), trace=trace, **kw)


def kernel(positions, hidden_states, w_qkv, q_norm_w, k_norm_w, w_o):
    in_maps = _host_inputs(positions, hidden_states, w_qkv, q_norm_w, k_norm_w, w_o)
    res = run_spmd(in_maps)
    return _assemble(res.results)


# revision 6
# speedup vs baseline: 1.1145x; 1.1145x over previous
"""Bass/Trainium2 kernel for nn_ExaoneMoEAttention (sliding-window GQA attention).

Strategy (8 NeuronCores, tensor-parallel over heads):
  - core c owns q heads 4c..4c+3 and kv head c (w_qkv column shard [4096, 768]),
    plus w_o rows 512c..512c+512 ([512, 4096]).
  - hidden is replicated (passed host-transposed as hidT [4096, 2048]).
  - QKV projection computed in [dim, t] layout (dim on partitions) with fp32r
    matmuls; per-head RMSNorm uses a ones-column matmul for the partition-dim
    reduction, RoPE uses host-precomputed cos/sin tables (halves duplicated).
  - Attention: scoresT[s, t] tiles of [128, 512]; sliding window (1024) +
    causal handled block-sparsely (<=12 key tiles per 512-wide q chunk) with
    multiplicative 0/1 masks; softmax without max-subtraction (RMSNorm bounds
    |score| <= sqrt(D)); exp-sum via ones-matmul; unnormalized attn @ v
    accumulated in PSUM; normalization by broadcast reciprocal.
  - o_proj per 512-row slab, then ReduceScatter(add) over the 8 cores per
    slab (overlaps with later slabs); host concatenates the 8 row-shards.
"""

import numpy as np

import concourse.bass as bass
import concourse.mybir as mybir
import concourse.tile as tile
from concourse import bacc
from concourse.bass_utils import run_bass_kernel_spmd
from concourse.masks import make_identity

F32 = mybir.dt.float32
F32R = mybir.dt.float32r
BF16 = mybir.dt.bfloat16
AF = mybir.ActivationFunctionType

N_CORES = 8
T = 2048
HID = 4096
H = 32
HKV = 8
D = 128
EPS = 1e-5
THETA = 1e6
WINDOW = 1024

HL = H // N_CORES          # 4 local q heads
CHUNK_A = 256              # t-chunk in QKV projection phase
N_CHUNKS_A = T // CHUNK_A  # 8
QC = 512                   # q chunk in attention phase
N_QC = T // QC             # 4
N_ST = T // 128            # 16 s-tiles
KO = HID // 128            # 32 k-subtiles in projection
ECH = 512                  # o_proj e-chunk
N_ECH = HID // ECH         # 8

# deltas (t_chunk_start - s_tile_start) that need masks, in host mask order
MASK_DELTAS = [0, -128, -256, -384, 640, 768, 896, 1024]
MASK_IDX = {d: i for i, d in enumerate(MASK_DELTAS)}


def _build():
    nc = bacc.Bacc(num_devices=N_CORES)

    hidT = nc.declare_dram_parameter("hidT", [128, N_CHUNKS_A, KO, CHUNK_A], F32R, isOutput=False)
    wq = nc.declare_dram_parameter("wq", [128, KO, (HL + 2) * D], F32R, isOutput=False)
    wo = nc.declare_dram_parameter("wo", [128, HL, HID], F32R, isOutput=False)
    cs2 = nc.declare_dram_parameter("cs2", [128, T], F32, isOutput=False)
    sn2s = nc.declare_dram_parameter("sn2s", [128, T], F32, isOutput=False)
    masks = nc.declare_dram_parameter("masks", [128, len(MASK_DELTAS), QC], F32, isOutput=False)
    qwv = nc.declare_dram_parameter("qwv", [D, 1], F32, isOutput=False)
    kwv = nc.declare_dram_parameter("kwv", [D, 1], F32, isOutput=False)
    onc_d = nc.declare_dram_parameter("onc", [128, 1], F32R, isOutput=False)
    onr_d = nc.declare_dram_parameter("onr", [1, 128], F32R, isOutput=False)
    out_p = nc.declare_dram_parameter("out", [N_QC, QC // N_CORES, HID], F32, isOutput=True)

    with tile.TileContext(nc) as tc:
        with (
            tc.tile_pool(name="persistA", bufs=1) as pA,
            tc.tile_pool(name="dramA", bufs=1, space="DRAM") as dramA,
        ):
            kT = pA.tile([128, T], F32R)           # rope'd k, [d, s]
            vnat = pA.tile([128, N_ST, D], F32R)   # v in [s, d] tiles
            onc = pA.tile([128, 1], F32R)
            onr = pA.tile([1, 128], F32R)
            nc.sync.dma_start(out=onc[:], in_=onc_d[:])
            nc.sync.dma_start(out=onr[:], in_=onr_d[:])

            qT_dram = [dramA.tile([128, T], F32R, name=f"qT{h}") for h in range(HL)]

            # ---------------- Phase A: QKV projection + norm + rope ----------
            with (
                tc.tile_pool(name="wpool", bufs=1) as wpool,
                tc.tile_pool(name="hidp", bufs=2) as hidp,
                tc.tile_pool(name="cspool", bufs=2) as cspool,
                tc.tile_pool(name="tmpA", bufs=2) as tmpA,
                tc.tile_pool(name="stA", bufs=2) as stA,
                tc.tile_pool(name="miscA", bufs=1) as miscA,
                tc.tile_pool(name="psq", bufs=2, space="PSUM") as psq_p,
                tc.tile_pool(name="psst", bufs=2, space="PSUM") as psst_p,
                tc.tile_pool(name="psbc", bufs=2, space="PSUM") as psbc_p,
                tc.tile_pool(name="psvt", bufs=2, space="PSUM") as psvt_p,
            ):
                w_grp = []
                for g in range(4):
                    wt = wpool.tile([128, KO // 4, (HL + 2) * D], F32R, name=f"w{g}")
                    nc.sync.dma_start(out=wt[:], in_=wq[:, g * (KO // 4):(g + 1) * (KO // 4), :])
                    w_grp.append(wt)
                qw_sb = miscA.tile([D, 1], F32)
                kw_sb = miscA.tile([D, 1], F32)
                ident = miscA.tile([128, 128], F32)
                eps_sb = miscA.tile([1, 1], F32)
                nc.sync.dma_start(out=qw_sb[:], in_=qwv[:])
                nc.sync.dma_start(out=kw_sb[:], in_=kwv[:])
                make_identity(nc, ident[:])
                nc.vector.memset(eps_sb[:], EPS)

                for tci in range(N_CHUNKS_A):
                    tsl = slice(tci * CHUNK_A, (tci + 1) * CHUNK_A)
                    hid_t = hidp.tile([128, KO, CHUNK_A], F32R, tag="hid")
                    nc.sync.dma_start(out=hid_t[:], in_=hidT[:, tci])
                    cs_t = cspool.tile([128, CHUNK_A], F32, tag="cs")
                    sn_t = cspool.tile([128, CHUNK_A], F32, tag="sn")
                    nc.sync.dma_start(out=cs_t[:], in_=cs2[:, tsl])
                    nc.sync.dma_start(out=sn_t[:], in_=sn2s[:, tsl])

                    for m in range(HL + 2):
                        psq = psq_p.tile([128, CHUNK_A], F32, tag="psq")
                        for ko in range(KO):
                            nc.tensor.matmul(
                                psq[:],
                                w_grp[ko // (KO // 4)][:, ko % (KO // 4), m * D:(m + 1) * D],
                                hid_t[:, ko, :],
                                start=(ko == 0),
                                stop=(ko == KO - 1),
                            )
                        if m < HL + 1:
                            # --- RMSNorm (partition-dim reduce via matmul) ---
                            sq = tmpA.tile([128, CHUNK_A], F32R, tag="sq")
                            nc.scalar.activation(sq[:], psq[:], AF.Square)
                            ssum = psst_p.tile([1, CHUNK_A], F32, tag="ssum")
                            nc.tensor.matmul(ssum[:], onc[:], sq[:], start=True, stop=True)
                            vtmp = stA.tile([1, CHUNK_A], F32, tag="vtmp")
                            nc.scalar.activation(
                                vtmp[:], ssum[:], AF.Sqrt, scale=1.0 / D, bias=eps_sb[:]
                            )
                            rstd = stA.tile([1, CHUNK_A], F32R, tag="rstd")
                            with nc.allow_low_precision(reason="f32r rstd for broadcast matmul"):
                                nc.vector.reciprocal(rstd[:], vtmp[:])
                            bcp = psbc_p.tile([128, CHUNK_A], F32, tag="bcp")
                            nc.tensor.matmul(bcp[:], onr[:], rstd[:], start=True, stop=True)
                            # apply norm weight (and softmax scale for q) + rstd
                            qn = tmpA.tile([128, CHUNK_A], F32, tag="qn")
                            nc.scalar.activation(
                                qn[:], psq[:], AF.Copy,
                                scale=(qw_sb[:] if m < HL else kw_sb[:]),
                            )
                            nc.vector.tensor_mul(qn[:], qn[:], bcp[:])
                            # --- RoPE (neox rotate-halves) ---
                            qsw = tmpA.tile([128, CHUNK_A], F32, tag="qsw")
                            nc.vector.tensor_copy(qsw[0:64, :], qn[64:128, :])
                            nc.vector.tensor_copy(qsw[64:128, :], qn[0:64, :])
                            nc.vector.tensor_mul(qn[:], qn[:], cs_t[:])
                            nc.vector.tensor_mul(qsw[:], qsw[:], sn_t[:])
                            if m < HL:
                                stg = tmpA.tile([128, CHUNK_A], F32R, tag="stg")
                                nc.vector.tensor_add(stg[:], qn[:], qsw[:])
                                nc.sync.dma_start(out=qT_dram[m][:, tsl], in_=stg[:])
                            else:
                                nc.vector.tensor_add(kT[:, tsl], qn[:], qsw[:])
                        else:
                            # --- v: copy + transpose into [s, d] tiles ---
                            vstg = tmpA.tile([128, CHUNK_A], F32, tag="vstg")
                            nc.scalar.activation(vstg[:], psq[:], AF.Copy)
                            for j in range(CHUNK_A // 128):
                                vt_ps = psvt_p.tile([128, 128], F32, tag="vt")
                                nc.tensor.transpose(
                                    vt_ps[:], vstg[:, j * 128:(j + 1) * 128], ident[:]
                                )
                                st_i = tci * (CHUNK_A // 128) + j
                                nc.vector.tensor_copy(vnat[:, st_i, :], vt_ps[:])

            # ---------------- Phase B: attention + o_proj + reduce-scatter ---
            with (
                tc.tile_pool(name="persistB", bufs=1) as pB,
                tc.tile_pool(name="qvp", bufs=3) as qvp,
                tc.tile_pool(name="exp", bufs=6) as exp_p,
                tc.tile_pool(name="stB", bufs=2) as stB,
                tc.tile_pool(name="ostg", bufs=3) as ostg_p,
                tc.tile_pool(name="cvp", bufs=1) as cvp,
                tc.tile_pool(name="psA", bufs=3, space="PSUM") as psA_p,
                tc.tile_pool(name="psav", bufs=2, space="PSUM") as psav_p,
                tc.tile_pool(name="pssum", bufs=2, space="PSUM") as pssum_p,
                tc.tile_pool(name="psbcB", bufs=1, space="PSUM") as psbcB_p,
                tc.tile_pool(name="dramB", bufs=1, space="DRAM") as dramB,
            ):
                attnT = pB.tile([128, HL, T], F32R)
                wo_sb = pB.tile([128, HL, HID], F32R)
                mask_sb = pB.tile([128, len(MASK_DELTAS), QC], F32)
                nc.sync.dma_start(out=wo_sb[:], in_=wo[:])
                nc.sync.dma_start(out=mask_sb[:], in_=masks[:])

                partial = [
                    dramB.tile([QC, HID], BF16, name=f"partial{qc}") for qc in range(N_QC)
                ]
                rs_out = [
                    dramB.tile([QC // N_CORES, HID], BF16, name=f"rsout{qc}")
                    for qc in range(N_QC)
                ]

                for qc in range(N_QC):
                    qsl = slice(qc * QC, (qc + 1) * QC)
                    si_lo = max(0, 4 * qc - 8)
                    si_hi = 4 * qc + 3
                    sis = list(range(si_lo, si_hi + 1))
                    for h in range(HL):
                        qv = qvp.tile([128, QC], F32R, tag="qv")
                        nc.sync.dma_start(out=qv[:], in_=qT_dram[h][:, qsl])
                        ps_av = psav_p.tile([128, QC], F32, tag="av")
                        ps_sum = pssum_p.tile([1, QC], F32, tag="sum")
                        exs = {}
                        # software-pipelined: scores run one s-tile ahead of
                        # the exp-consuming matmuls
                        def emit_scores(si):
                            psc = psA_p.tile([128, QC], F32, tag="sc")
                            nc.tensor.matmul(
                                psc[:], kT[:, si * 128:(si + 1) * 128], qv[:],
                                start=True, stop=True,
                            )
                            ex = exp_p.tile([128, QC], F32R, tag="ex")
                            nc.scalar.activation(ex[:], psc[:], AF.Exp)
                            delta = qc * QC - si * 128
                            if delta in MASK_IDX:
                                nc.vector.tensor_mul(
                                    ex[:], ex[:], mask_sb[:, MASK_IDX[delta], :]
                                )
                            exs[si] = ex

                        def emit_consume(si):
                            ex = exs.pop(si)
                            first = si == sis[0]
                            last = si == sis[-1]
                            nc.tensor.matmul(
                                ps_sum[:], onc[:], ex[:], start=first, stop=last
                            )
                            nc.tensor.matmul(
                                ps_av[:], vnat[:, si, :], ex[:], start=first, stop=last
                            )

                        emit_scores(sis[0])
                        for si in sis[1:]:
                            emit_scores(si)
                            emit_consume(si - 1)
                        emit_consume(sis[-1])

                        rc = stB.tile([1, QC], F32R, tag="rc")
                        with nc.allow_low_precision(reason="f32r recip for broadcast matmul"):
                            nc.vector.reciprocal(rc[:], ps_sum[:])
                        bcp = psbcB_p.tile([128, QC], F32, tag="bcB")
                        nc.tensor.matmul(bcp[:], onr[:], rc[:], start=True, stop=True)
                        at = attnT[:, h, qsl]
                        nc.scalar.activation(at, ps_av[:], AF.Copy)
                        nc.vector.tensor_mul(at, at, bcp[:])

                    # o_proj for this 512-row slab
                    for tt in range(QC // 128):
                        trow = qc * 4 + tt
                        for ec in range(N_ECH):
                            pso = psA_p.tile([128, ECH], F32, tag="sc")
                            for h in range(HL):
                                nc.tensor.matmul(
                                    pso[:],
                                    attnT[:, h, trow * 128:(trow + 1) * 128],
                                    wo_sb[:, h, ec * ECH:(ec + 1) * ECH],
                                    start=(h == 0),
                                    stop=(h == HL - 1),
                                )
                            ost = ostg_p.tile([128, ECH], BF16, tag="ost")
                            nc.any.tensor_copy(out=ost[:], in_=pso[:])
                            nc.sync.dma_start(
                                out=partial[qc][tt * 128:(tt + 1) * 128,
                                                ec * ECH:(ec + 1) * ECH],
                                in_=ost[:],
                            )
                    nc.gpsimd.collective_compute(
                        "ReduceScatter",
                        mybir.AluOpType.add,
                        replica_groups=[list(range(N_CORES))],
                        ins=[partial[qc][:]],
                        outs=[rs_out[qc][:]],
                    )
                    cv_b = cvp.tile([QC // N_CORES, HID], BF16, tag="cvb")
                    cv_f = cvp.tile([QC // N_CORES, HID], F32, tag="cvf")
                    nc.sync.dma_start(out=cv_b[:], in_=rs_out[qc][:])
                    nc.vector.tensor_copy(cv_f[:], cv_b[:])
                    nc.sync.dma_start(out=out_p[qc], in_=cv_f[:])

    nc.finalize()
    return nc


_NC_CACHE = None


def _get_nc():
    global _NC_CACHE
    if _NC_CACHE is None:
        _NC_CACHE = _build()
    return _NC_CACHE


def _host_inputs(positions, hidden_states, w_qkv, q_norm_w, k_norm_w, w_o):
    positions = np.asarray(positions)
    hidden_states = np.asarray(hidden_states, dtype=np.float32)
    w_qkv = np.asarray(w_qkv, dtype=np.float32)
    q_norm_w = np.asarray(q_norm_w, dtype=np.float32)
    k_norm_w = np.asarray(k_norm_w, dtype=np.float32)
    w_o = np.asarray(w_o, dtype=np.float32)

    # [ki, tc, ko, j]: hidT4[ki, tc, ko, j] = hidden[tc*CHUNK_A+j, ko*128+ki]
    hidT4 = np.ascontiguousarray(
        hidden_states.T.reshape(KO, 128, N_CHUNKS_A, CHUNK_A).transpose(1, 2, 0, 3)
    )

    half = D // 2
    inv_freq = 1.0 / (THETA ** (np.arange(half, dtype=np.float32) / half))
    ang = positions.astype(np.float32)[:, None] * inv_freq[None, :]  # [T, 64]
    cos = np.cos(ang).T.astype(np.float32)   # [64, T]
    sin = np.sin(ang).T.astype(np.float32)
    cs2 = np.concatenate([cos, cos], axis=0)          # [128, T]
    sn2s = np.concatenate([-sin, sin], axis=0)        # [128, T]

    mk = np.zeros((len(MASK_DELTAS), 128, QC), np.float32)
    ss = np.arange(128)[:, None]
    ttv = np.arange(QC)[None, :]
    for i, dlt in enumerate(MASK_DELTAS):
        diff = dlt + ttv - ss
        mk[i] = ((diff >= 0) & (diff < WINDOW)).astype(np.float32)
    mk = np.ascontiguousarray(mk.transpose(1, 0, 2))  # [128, nmask, QC]

    qwv = (q_norm_w * (D ** -0.5)).reshape(D, 1).astype(np.float32)
    kwv = k_norm_w.reshape(D, 1).astype(np.float32)
    onc = np.ones((128, 1), np.float32)
    onr = np.ones((1, 128), np.float32)

    in_maps = []
    for c in range(N_CORES):
        wq_c = np.concatenate(
            [
                w_qkv[:, c * HL * D:(c + 1) * HL * D],
                w_qkv[:, H * D + c * D:H * D + (c + 1) * D],
                w_qkv[:, (H + HKV) * D + c * D:(H + HKV) * D + (c + 1) * D],
            ],
            axis=1,
        )
        # [ki, ko, m]
        wq_c = np.ascontiguousarray(wq_c.reshape(KO, 128, (HL + 2) * D).transpose(1, 0, 2))
        # [p, h, e]
        wo_c = np.ascontiguousarray(
            w_o[c * HL * D:(c + 1) * HL * D, :].reshape(HL, 128, HID).transpose(1, 0, 2)
        )
        in_maps.append(
            {
                "hidT": hidT4,
                "wq": wq_c,
                "wo": wo_c,
                "cs2": cs2,
                "sn2s": sn2s,
                "masks": mk,
                "qwv": qwv,
                "kwv": kwv,
                "onc": onc,
                "onr": onr,
            }
        )
    return in_maps


def _assemble(results):
    out = np.empty((T, HID), np.float32)
    rows = QC // N_CORES
    for c in range(N_CORES):
        r = results[c]["out"]  # [N_QC, rows, HID]
        for qc in range(N_QC):
            out[qc * QC + c * rows: qc * QC + (c + 1) * rows] = r[qc]
    return out


def run_spmd(in_maps, trace=False, **kw):
    nc = _get_nc()
    return run_bass_kernel_spmd(nc, in_maps, list(range(N_CORES)), trace=trace, **kw)


def kernel(positions, hidden_states, w_qkv, q_norm_w, k_norm_w, w_o):
    in_maps = _host_inputs(positions, hidden_states, w_qkv, q_norm_w, k_norm_w, w_o)
    res = run_spmd(in_maps)
    return _assemble(res.results)


# revision 8
# speedup vs baseline: 1.1244x; 1.0089x over previous
"""Bass/Trainium2 kernel for nn_ExaoneMoEAttention (sliding-window GQA attention).

Strategy (8 NeuronCores, tensor-parallel over heads):
  - core c owns q heads 4c..4c+3 and kv head c (w_qkv column shard [4096, 768]),
    plus w_o rows 512c..512c+512 ([512, 4096]).
  - hidden is replicated (passed host-transposed as hidT [4096, 2048]).
  - QKV projection computed in [dim, t] layout (dim on partitions) with fp32r
    matmuls; per-head RMSNorm uses a ones-column matmul for the partition-dim
    reduction, RoPE uses host-precomputed cos/sin tables (halves duplicated).
  - Attention: scoresT[s, t] tiles of [128, 512]; sliding window (1024) +
    causal handled block-sparsely (<=12 key tiles per 512-wide q chunk) with
    multiplicative 0/1 masks; softmax without max-subtraction (RMSNorm bounds
    |score| <= sqrt(D)); exp-sum via ones-matmul; unnormalized attn @ v
    accumulated in PSUM; normalization by broadcast reciprocal.
  - o_proj per 512-row slab, then ReduceScatter(add) over the 8 cores per
    slab (overlaps with later slabs); host concatenates the 8 row-shards.
"""

import numpy as np

import concourse.bass as bass
import concourse.mybir as mybir
import concourse.tile as tile
from concourse import bacc
from concourse.bass_utils import run_bass_kernel_spmd
from concourse.masks import make_identity

F32 = mybir.dt.float32
F32R = mybir.dt.float32r
BF16 = mybir.dt.bfloat16
AF = mybir.ActivationFunctionType

N_CORES = 8
T = 2048
HID = 4096
H = 32
HKV = 8
D = 128
EPS = 1e-5
THETA = 1e6
WINDOW = 1024

HL = H // N_CORES          # 4 local q heads
CHUNK_A = 256              # t-chunk in QKV projection phase
N_CHUNKS_A = T // CHUNK_A  # 8
QC = 512                   # q chunk in attention phase
N_QC = T // QC             # 4
N_ST = T // 128            # 16 s-tiles
KO = HID // 128            # 32 k-subtiles in projection
ECH = 512                  # o_proj e-chunk
N_ECH = HID // ECH         # 8

# deltas (t_chunk_start - s_tile_start) that need masks, in host mask order
MASK_DELTAS = [0, -128, -256, -384, 640, 768, 896, 1024]
MASK_IDX = {d: i for i, d in enumerate(MASK_DELTAS)}


def _build():
    nc = bacc.Bacc(num_devices=N_CORES)

    hidT = nc.declare_dram_parameter("hidT", [128, N_CHUNKS_A, KO, CHUNK_A], F32R, isOutput=False)
    wq = nc.declare_dram_parameter("wq", [128, KO, (HL + 2) * D], F32R, isOutput=False)
    wo = nc.declare_dram_parameter("wo", [128, HL, HID], F32R, isOutput=False)
    cs2 = nc.declare_dram_parameter("cs2", [128, T], F32, isOutput=False)
    sn2s = nc.declare_dram_parameter("sn2s", [128, T], F32, isOutput=False)
    masks = nc.declare_dram_parameter("masks", [128, len(MASK_DELTAS), QC], F32, isOutput=False)
    qwv = nc.declare_dram_parameter("qwv", [D, 1], F32, isOutput=False)
    kwv = nc.declare_dram_parameter("kwv", [D, 1], F32, isOutput=False)
    onc_d = nc.declare_dram_parameter("onc", [128, 1], F32R, isOutput=False)
    onr_d = nc.declare_dram_parameter("onr", [1, 128], F32R, isOutput=False)
    out_p = nc.declare_dram_parameter("out", [N_QC, QC // N_CORES, HID], F32, isOutput=True)

    with tile.TileContext(nc) as tc:
        with (
            tc.tile_pool(name="persistA", bufs=1) as pA,
            tc.tile_pool(name="dramA", bufs=1, space="DRAM") as dramA,
        ):
            kT = pA.tile([128, T], F32R)           # rope'd k, [d, s]
            vnat = pA.tile([128, N_ST, D], F32R)   # v in [s, d] tiles
            onc = pA.tile([128, 1], F32R)
            onr = pA.tile([1, 128], F32R)
            nc.sync.dma_start(out=onc[:], in_=onc_d[:])
            nc.sync.dma_start(out=onr[:], in_=onr_d[:])

            qT_dram = [dramA.tile([128, T], F32R, name=f"qT{h}") for h in range(HL)]

            # ---------------- Phase A: QKV projection + norm + rope ----------
            with (
                tc.tile_pool(name="wpool", bufs=1) as wpool,
                tc.tile_pool(name="hidp", bufs=2) as hidp,
                tc.tile_pool(name="cspool", bufs=2) as cspool,
                tc.tile_pool(name="tmpA", bufs=2) as tmpA,
                tc.tile_pool(name="stA", bufs=2) as stA,
                tc.tile_pool(name="miscA", bufs=1) as miscA,
                tc.tile_pool(name="psq", bufs=2, space="PSUM") as psq_p,
                tc.tile_pool(name="psst", bufs=2, space="PSUM") as psst_p,
                tc.tile_pool(name="psbc", bufs=2, space="PSUM") as psbc_p,
                tc.tile_pool(name="psvt", bufs=2, space="PSUM") as psvt_p,
            ):
                w_grp = []
                for g in range(4):
                    wt = wpool.tile([128, KO // 4, (HL + 2) * D], F32R, name=f"w{g}")
                    nc.sync.dma_start(out=wt[:], in_=wq[:, g * (KO // 4):(g + 1) * (KO // 4), :])
                    w_grp.append(wt)
                qw_sb = miscA.tile([D, 1], F32)
                kw_sb = miscA.tile([D, 1], F32)
                ident = miscA.tile([128, 128], F32)
                eps_sb = miscA.tile([1, 1], F32)
                nc.sync.dma_start(out=qw_sb[:], in_=qwv[:])
                nc.sync.dma_start(out=kw_sb[:], in_=kwv[:])
                make_identity(nc, ident[:])
                nc.vector.memset(eps_sb[:], EPS)

                for tci in range(N_CHUNKS_A):
                    tsl = slice(tci * CHUNK_A, (tci + 1) * CHUNK_A)
                    hid_t = hidp.tile([128, KO, CHUNK_A], F32R, tag="hid")
                    nc.sync.dma_start(out=hid_t[:], in_=hidT[:, tci])
                    cs_t = cspool.tile([128, CHUNK_A], F32, tag="cs")
                    sn_t = cspool.tile([128, CHUNK_A], F32, tag="sn")
                    nc.sync.dma_start(out=cs_t[:], in_=cs2[:, tsl])
                    nc.sync.dma_start(out=sn_t[:], in_=sn2s[:, tsl])

                    for m in range(HL + 2):
                        psq = psq_p.tile([128, CHUNK_A], F32, tag="psq")
                        for ko in range(KO):
                            nc.tensor.matmul(
                                psq[:],
                                w_grp[ko // (KO // 4)][:, ko % (KO // 4), m * D:(m + 1) * D],
                                hid_t[:, ko, :],
                                start=(ko == 0),
                                stop=(ko == KO - 1),
                            )
                        if m < HL + 1:
                            # --- RMSNorm (partition-dim reduce via matmul) ---
                            sq = tmpA.tile([128, CHUNK_A], F32R, tag="sq")
                            nc.scalar.activation(sq[:], psq[:], AF.Square)
                            ssum = psst_p.tile([1, CHUNK_A], F32, tag="ssum")
                            nc.tensor.matmul(ssum[:], onc[:], sq[:], start=True, stop=True)
                            vtmp = stA.tile([1, CHUNK_A], F32, tag="vtmp")
                            nc.scalar.activation(
                                vtmp[:], ssum[:], AF.Sqrt, scale=1.0 / D, bias=eps_sb[:]
                            )
                            rstd = stA.tile([1, CHUNK_A], F32R, tag="rstd")
                            with nc.allow_low_precision(reason="f32r rstd for broadcast matmul"):
                                nc.vector.reciprocal(rstd[:], vtmp[:])
                            bcp = psbc_p.tile([128, CHUNK_A], F32, tag="bcp")
                            nc.tensor.matmul(bcp[:], onr[:], rstd[:], start=True, stop=True)
                            # apply norm weight (and softmax scale for q) + rstd
                            qn = tmpA.tile([128, CHUNK_A], F32, tag="qn")
                            nc.scalar.activation(
                                qn[:], psq[:], AF.Copy,
                                scale=(qw_sb[:] if m < HL else kw_sb[:]),
                            )
                            nc.vector.tensor_mul(qn[:], qn[:], bcp[:])
                            # --- RoPE (neox rotate-halves) ---
                            qsw = tmpA.tile([128, CHUNK_A], F32, tag="qsw")
                            nc.vector.tensor_copy(qsw[0:64, :], qn[64:128, :])
                            nc.vector.tensor_copy(qsw[64:128, :], qn[0:64, :])
                            nc.vector.tensor_mul(qn[:], qn[:], cs_t[:])
                            nc.vector.tensor_mul(qsw[:], qsw[:], sn_t[:])
                            if m < HL:
                                stg = tmpA.tile([128, CHUNK_A], F32R, tag="stg")
                                nc.vector.tensor_add(stg[:], qn[:], qsw[:])
                                nc.sync.dma_start(out=qT_dram[m][:, tsl], in_=stg[:])
                            else:
                                nc.vector.tensor_add(kT[:, tsl], qn[:], qsw[:])
                        else:
                            # --- v: copy + transpose into [s, d] tiles ---
                            vstg = tmpA.tile([128, CHUNK_A], F32, tag="vstg")
                            nc.scalar.activation(vstg[:], psq[:], AF.Copy)
                            for j in range(CHUNK_A // 128):
                                vt_ps = psvt_p.tile([128, 128], F32, tag="vt")
                                nc.tensor.transpose(
                                    vt_ps[:], vstg[:, j * 128:(j + 1) * 128], ident[:]
                                )
                                st_i = tci * (CHUNK_A // 128) + j
                                nc.vector.tensor_copy(vnat[:, st_i, :], vt_ps[:])

            # ---------------- Phase B: attention + o_proj + reduce-scatter ---
            with (
                tc.tile_pool(name="persistB", bufs=1) as pB,
                tc.tile_pool(name="qvp", bufs=3) as qvp,
                tc.tile_pool(name="exp", bufs=6) as exp_p,
                tc.tile_pool(name="stB", bufs=2) as stB,
                tc.tile_pool(name="ostg", bufs=3) as ostg_p,
                tc.tile_pool(name="cvp", bufs=1) as cvp,
                tc.tile_pool(name="psA", bufs=3, space="PSUM") as psA_p,
                tc.tile_pool(name="psav", bufs=2, space="PSUM") as psav_p,
                tc.tile_pool(name="pssum", bufs=2, space="PSUM") as pssum_p,
                tc.tile_pool(name="psbcB", bufs=1, space="PSUM") as psbcB_p,
                tc.tile_pool(name="dramB", bufs=1, space="DRAM") as dramB,
            ):
                attnT = pB.tile([128, HL, T], F32R)
                wo_sb = pB.tile([128, HL, HID], F32R)
                mask_sb = pB.tile([128, len(MASK_DELTAS), QC], F32)
                nc.sync.dma_start(out=wo_sb[:], in_=wo[:])
                nc.sync.dma_start(out=mask_sb[:], in_=masks[:])

                partial = [
                    dramB.tile([QC, HID], BF16, name=f"partial{qc}") for qc in range(N_QC)
                ]
                rs_out = [
                    dramB.tile([QC // N_CORES, HID], BF16, name=f"rsout{qc}")
                    for qc in range(N_QC)
                ]

                for qc in range(N_QC):
                    qsl = slice(qc * QC, (qc + 1) * QC)
                    si_lo = max(0, 4 * qc - 8)
                    si_hi = 4 * qc + 3
                    sis = list(range(si_lo, si_hi + 1))
                    for hp in range(0, HL, 2):
                        qvs, avs, sums = [], [], []
                        for h in (hp, hp + 1):
                            qv = qvp.tile([128, QC], F32R, tag="qv")
                            nc.sync.dma_start(out=qv[:], in_=qT_dram[h][:, qsl])
                            qvs.append(qv)
                            avs.append(psav_p.tile([128, QC], F32, tag="av", name="av"))
                            sums.append(pssum_p.tile([1, QC], F32, tag="sum", name="sum"))
                        exs = {}

                        # two heads share kT/vnat tiles; their independent
                        # score->exp->consume chains interleave so ACT/DVE
                        # latency hides under PE work
                        def emit_scores(si):
                            for j in range(2):
                                psc = psA_p.tile([128, QC], F32, tag="sc")
                                nc.tensor.matmul(
                                    psc[:], kT[:, si * 128:(si + 1) * 128],
                                    qvs[j][:], start=True, stop=True,
                                )
                                ex = exp_p.tile([128, QC], F32R, tag="ex")
                                nc.scalar.activation(ex[:], psc[:], AF.Exp)
                                delta = qc * QC - si * 128
                                if delta in MASK_IDX:
                                    nc.vector.tensor_mul(
                                        ex[:], ex[:], mask_sb[:, MASK_IDX[delta], :]
                                    )
                                exs[(si, j)] = ex

                        def emit_consume(si):
                            first = si == sis[0]
                            last = si == sis[-1]
                            for j in range(2):
                                ex = exs.pop((si, j))
                                nc.tensor.matmul(
                                    sums[j][:], onc[:], ex[:], start=first, stop=last
                                )
                                nc.tensor.matmul(
                                    avs[j][:], vnat[:, si, :], ex[:], start=first, stop=last
                                )

                        emit_scores(sis[0])
                        for si in sis[1:]:
                            emit_scores(si)
                            emit_consume(si - 1)
                        emit_consume(sis[-1])

                        for j in range(2):
                            rc = stB.tile([1, QC], F32R, tag="rc")
                            with nc.allow_low_precision(reason="f32r recip for broadcast matmul"):
                                nc.vector.reciprocal(rc[:], sums[j][:])
                            bcp = psbcB_p.tile([128, QC], F32, tag="bcB")
                            nc.tensor.matmul(bcp[:], onr[:], rc[:], start=True, stop=True)
                            at = attnT[:, hp + j, qsl]
                            nc.scalar.activation(at, avs[j][:], AF.Copy)
                            nc.vector.tensor_mul(at, at, bcp[:])

                    # o_proj for this 512-row slab
                    for tt in range(QC // 128):
                        trow = qc * 4 + tt
                        for ec in range(N_ECH):
                            pso = psA_p.tile([128, ECH], F32, tag="sc")
                            for h in range(HL):
                                nc.tensor.matmul(
                                    pso[:],
                                    attnT[:, h, trow * 128:(trow + 1) * 128],
                                    wo_sb[:, h, ec * ECH:(ec + 1) * ECH],
                                    start=(h == 0),
                                    stop=(h == HL - 1),
                                )
                            ost = ostg_p.tile([128, ECH], BF16, tag="ost")
                            nc.any.tensor_copy(out=ost[:], in_=pso[:])
                            nc.sync.dma_start(
                                out=partial[qc][tt * 128:(tt + 1) * 128,
                                                ec * ECH:(ec + 1) * ECH],
                                in_=ost[:],
                            )
                    nc.gpsimd.collective_compute(
                        "ReduceScatter",
                        mybir.AluOpType.add,
                        replica_groups=[list(range(N_CORES))],
                        ins=[partial[qc][:]],
                        outs=[rs_out[qc][:]],
                    )
                    cv_b = cvp.tile([QC // N_CORES, HID], BF16, tag="cvb")
                    cv_f = cvp.tile([QC // N_CORES, HID], F32, tag="cvf")
                    nc.sync.dma_start(out=cv_b[:], in_=rs_out[qc][:])
                    nc.vector.tensor_copy(cv_f[:], cv_b[:])
                    nc.sync.dma_start(out=out_p[qc], in_=cv_f[:])

    nc.finalize()
    return nc


_NC_CACHE = None


def _get_nc():
    global _NC_CACHE
    if _NC_CACHE is None:
        _NC_CACHE = _build()
    return _NC_CACHE


def _host_inputs(positions, hidden_states, w_qkv, q_norm_w, k_norm_w, w_o):
    positions = np.asarray(positions)
    hidden_states = np.asarray(hidden_states, dtype=np.float32)
    w_qkv = np.asarray(w_qkv, dtype=np.float32)
    q_norm_w = np.asarray(q_norm_w, dtype=np.float32)
    k_norm_w = np.asarray(k_norm_w, dtype=np.float32)
    w_o = np.asarray(w_o, dtype=np.float32)

    # [ki, tc, ko, j]: hidT4[ki, tc, ko, j] = hidden[tc*CHUNK_A+j, ko*128+ki]
    hidT4 = np.ascontiguousarray(
        hidden_states.T.reshape(KO, 128, N_CHUNKS_A, CHUNK_A).transpose(1, 2, 0, 3)
    )

    half = D // 2
    inv_freq = 1.0 / (THETA ** (np.arange(half, dtype=np.float32) / half))
    ang = positions.astype(np.float32)[:, None] * inv_freq[None, :]  # [T, 64]
    cos = np.cos(ang).T.astype(np.float32)   # [64, T]
    sin = np.sin(ang).T.astype(np.float32)
    cs2 = np.concatenate([cos, cos], axis=0)          # [128, T]
    sn2s = np.concatenate([-sin, sin], axis=0)        # [128, T]

    mk = np.zeros((len(MASK_DELTAS), 128, QC), np.float32)
    ss = np.arange(128)[:, None]
    ttv = np.arange(QC)[None, :]
    for i, dlt in enumerate(MASK_DELTAS):
        diff = dlt + ttv - ss
        mk[i] = ((diff >= 0) & (diff < WINDOW)).astype(np.float32)
    mk = np.ascontiguousarray(mk.transpose(1, 0, 2))  # [128, nmask, QC]

    qwv = (q_norm_w * (D ** -0.5)).reshape(D, 1).astype(np.float32)
    kwv = k_norm_w.reshape(D, 1).astype(np.float32)
    onc = np.ones((128, 1), np.float32)
    onr = np.ones((1, 128), np.float32)

    in_maps = []
    for c in range(N_CORES):
        wq_c = np.concatenate(
            [
                w_qkv[:, c * HL * D:(c + 1) * HL * D],
                w_qkv[:, H * D + c * D:H * D + (c + 1) * D],
                w_qkv[:, (H + HKV) * D + c * D:(H + HKV) * D + (c + 1) * D],
            ],
            axis=1,
        )
        # [ki, ko, m]
        wq_c = np.ascontiguousarray(wq_c.reshape(KO, 128, (HL + 2) * D).transpose(1, 0, 2))
        # [p, h, e]
        wo_c = np.ascontiguousarray(
            w_o[c * HL * D:(c + 1) * HL * D, :].reshape(HL, 128, HID).transpose(1, 0, 2)
        )
        in_maps.append(
            {
                "hidT": hidT4,
                "wq": wq_c,
                "wo": wo_c,
                "cs2": cs2,
                "sn2s": sn2s,
                "masks": mk,
                "qwv": qwv,
                "kwv": kwv,
                "onc": onc,
                "onr": onr,
            }
        )
    return in_maps


def _assemble(results):
    out = np.empty((T, HID), np.float32)
    rows = QC // N_CORES
    for c in range(N_CORES):
        r = results[c]["out"]  # [N_QC, rows, HID]
        for qc in range(N_QC):
            out[qc * QC + c * rows: qc * QC + (c + 1) * rows] = r[qc]
    return out


def run_spmd(in_maps, trace=False, **kw):
    nc = _get_nc()
    return run_bass_kernel_spmd(nc, in_maps, list(range(N_CORES)), trace=trace, **kw)


def kernel(positions, hidden_states, w_qkv, q_norm_w, k_norm_w, w_o):
    in_maps = _host_inputs(positions, hidden_states, w_qkv, q_norm_w, k_norm_w, w_o)
    res = run_spmd(in_maps)
    return _assemble(res.results)


# revision 10
# speedup vs baseline: 1.1477x; 1.0207x over previous
"""Bass/Trainium2 kernel for nn_ExaoneMoEAttention (sliding-window GQA attention).

Strategy (8 NeuronCores, tensor-parallel over heads):
  - core c owns q heads 4c..4c+3 and kv head c (w_qkv column shard [4096, 768]),
    plus w_o rows 512c..512c+512 ([512, 4096]).
  - hidden is replicated (passed host-transposed as hidT [4096, 2048]).
  - QKV projection computed in [dim, t] layout (dim on partitions) with fp32r
    matmuls; per-head RMSNorm uses a ones-column matmul for the partition-dim
    reduction, RoPE uses host-precomputed cos/sin tables (halves duplicated).
  - Attention: scoresT[s, t] tiles of [128, 512]; sliding window (1024) +
    causal handled block-sparsely (<=12 key tiles per 512-wide q chunk) with
    multiplicative 0/1 masks; softmax without max-subtraction (RMSNorm bounds
    |score| <= sqrt(D)); exp-sum via ones-matmul; unnormalized attn @ v
    accumulated in PSUM; normalization by broadcast reciprocal.
  - o_proj per 512-row slab, then ReduceScatter(add) over the 8 cores per
    slab (overlaps with later slabs); host concatenates the 8 row-shards.
"""

import numpy as np

import concourse.bass as bass
import concourse.mybir as mybir
import concourse.tile as tile
from concourse import bacc
from concourse.bass_utils import run_bass_kernel_spmd
from concourse.masks import make_identity

F32 = mybir.dt.float32
F32R = mybir.dt.float32r
BF16 = mybir.dt.bfloat16
AF = mybir.ActivationFunctionType

N_CORES = 8
T = 2048
HID = 4096
H = 32
HKV = 8
D = 128
EPS = 1e-5
THETA = 1e6
WINDOW = 1024

HL = H // N_CORES          # 4 local q heads
CHUNK_A = 256              # t-chunk in QKV projection phase
N_CHUNKS_A = T // CHUNK_A  # 8
QC = 512                   # q chunk in attention phase
N_QC = T // QC             # 4
N_ST = T // 128            # 16 s-tiles
KO = HID // 128            # 32 k-subtiles in projection
ECH = 512                  # o_proj e-chunk
N_ECH = HID // ECH         # 8

# deltas (t_chunk_start - s_tile_start) that need masks, in host mask order
MASK_DELTAS = [0, -128, -256, -384, 640, 768, 896, 1024]
MASK_IDX = {d: i for i, d in enumerate(MASK_DELTAS)}


def _build():
    nc = bacc.Bacc(num_devices=N_CORES)

    hidT = nc.declare_dram_parameter("hidT", [128, N_CHUNKS_A, KO, CHUNK_A], F32R, isOutput=False)
    wq = nc.declare_dram_parameter("wq", [128, KO, (HL + 2) * D], F32R, isOutput=False)
    wo = nc.declare_dram_parameter("wo", [128, HL, HID], F32R, isOutput=False)
    cs2 = nc.declare_dram_parameter("cs2", [128, T], F32, isOutput=False)
    sn2s = nc.declare_dram_parameter("sn2s", [128, T], F32, isOutput=False)
    masks = nc.declare_dram_parameter("masks", [128, len(MASK_DELTAS), QC], F32, isOutput=False)
    qwv = nc.declare_dram_parameter("qwv", [D, 1], F32, isOutput=False)
    kwv = nc.declare_dram_parameter("kwv", [D, 1], F32, isOutput=False)
    onc_d = nc.declare_dram_parameter("onc", [128, 1], F32R, isOutput=False)
    onr_d = nc.declare_dram_parameter("onr", [1, 128], F32R, isOutput=False)
    out_p = nc.declare_dram_parameter("out", [N_QC, QC // N_CORES, HID], F32, isOutput=True)

    with tile.TileContext(nc) as tc:
        with (
            tc.tile_pool(name="persistA", bufs=1) as pA,
            tc.tile_pool(name="dramA", bufs=1, space="DRAM") as dramA,
        ):
            kT = pA.tile([128, T], F32R)           # rope'd k, [d, s]
            vnat = pA.tile([128, N_ST, D], F32R)   # v in [s, d] tiles
            onc = pA.tile([128, 1], F32R)
            onr = pA.tile([1, 128], F32R)
            nc.sync.dma_start(out=onc[:], in_=onc_d[:])
            nc.sync.dma_start(out=onr[:], in_=onr_d[:])

            qT_dram = [dramA.tile([128, T], F32R, name=f"qT{h}") for h in range(HL)]

            # ---------------- Phase A: QKV projection + norm + rope ----------
            with (
                tc.tile_pool(name="wpool", bufs=1) as wpool,
                tc.tile_pool(name="hidp", bufs=2) as hidp,
                tc.tile_pool(name="cspool", bufs=2) as cspool,
                tc.tile_pool(name="tmpA", bufs=2) as tmpA,
                tc.tile_pool(name="stA", bufs=2) as stA,
                tc.tile_pool(name="miscA", bufs=1) as miscA,
                tc.tile_pool(name="psq", bufs=2, space="PSUM") as psq_p,
                tc.tile_pool(name="psst", bufs=2, space="PSUM") as psst_p,
                tc.tile_pool(name="psbc", bufs=2, space="PSUM") as psbc_p,
                tc.tile_pool(name="psvt", bufs=2, space="PSUM") as psvt_p,
            ):
                w_grp = []
                for g in range(4):
                    wt = wpool.tile([128, KO // 4, (HL + 2) * D], F32R, name=f"w{g}")
                    nc.sync.dma_start(out=wt[:], in_=wq[:, g * (KO // 4):(g + 1) * (KO // 4), :])
                    w_grp.append(wt)
                qw_sb = miscA.tile([D, 1], F32)
                kw_sb = miscA.tile([D, 1], F32)
                ident = miscA.tile([128, 128], F32)
                eps_sb = miscA.tile([1, 1], F32)
                nc.sync.dma_start(out=qw_sb[:], in_=qwv[:])
                nc.sync.dma_start(out=kw_sb[:], in_=kwv[:])
                make_identity(nc, ident[:])
                nc.vector.memset(eps_sb[:], EPS)

                pending_post = [None]

                def flush_post():
                    if pending_post[0] is not None:
                        pending_post[0]()
                        pending_post[0] = None

                for tci in range(N_CHUNKS_A):
                    tsl = slice(tci * CHUNK_A, (tci + 1) * CHUNK_A)
                    hid_t = hidp.tile([128, KO, CHUNK_A], F32R, tag="hid")
                    nc.sync.dma_start(out=hid_t[:], in_=hidT[:, tci])
                    cs_t = cspool.tile([128, CHUNK_A], F32, tag="cs")
                    sn_t = cspool.tile([128, CHUNK_A], F32, tag="sn")
                    nc.sync.dma_start(out=cs_t[:], in_=cs2[:, tsl])
                    nc.sync.dma_start(out=sn_t[:], in_=sn2s[:, tsl])

                    for m in range(HL + 2):
                        psq = psq_p.tile([128, CHUNK_A], F32, tag="psq")
                        for ko in range(KO):
                            nc.tensor.matmul(
                                psq[:],
                                w_grp[ko // (KO // 4)][:, ko % (KO // 4), m * D:(m + 1) * D],
                                hid_t[:, ko, :],
                                start=(ko == 0),
                                stop=(ko == KO - 1),
                            )
                        flush_post()

                        def make_post(m=m, psq=psq, tsl=tsl, tci=tci, cs_t=cs_t, sn_t=sn_t):
                            def _post():
                                if m < HL + 1:
                                    # RMSNorm (partition-dim reduce via matmul)
                                    sq = tmpA.tile([128, CHUNK_A], F32R, tag="sq")
                                    nc.scalar.activation(sq[:], psq[:], AF.Square)
                                    ssum = psst_p.tile([1, CHUNK_A], F32, tag="ssum")
                                    nc.tensor.matmul(ssum[:], onc[:], sq[:], start=True, stop=True)
                                    vtmp = stA.tile([1, CHUNK_A], F32, tag="vtmp")
                                    nc.scalar.activation(
                                        vtmp[:], ssum[:], AF.Sqrt, scale=1.0 / D, bias=eps_sb[:]
                                    )
                                    rstd = stA.tile([1, CHUNK_A], F32R, tag="rstd")
                                    with nc.allow_low_precision(reason="f32r rstd"):
                                        nc.vector.reciprocal(rstd[:], vtmp[:])
                                    bcp = psbc_p.tile([128, CHUNK_A], F32, tag="bcp")
                                    nc.tensor.matmul(bcp[:], onr[:], rstd[:], start=True, stop=True)
                                    qn = tmpA.tile([128, CHUNK_A], F32, tag="qn")
                                    nc.scalar.activation(
                                        qn[:], psq[:], AF.Copy,
                                        scale=(qw_sb[:] if m < HL else kw_sb[:]),
                                    )
                                    nc.vector.tensor_mul(qn[:], qn[:], bcp[:])
                                    # RoPE (neox rotate-halves)
                                    qsw = tmpA.tile([128, CHUNK_A], F32, tag="qsw")
                                    nc.vector.tensor_copy(qsw[0:64, :], qn[64:128, :])
                                    nc.vector.tensor_copy(qsw[64:128, :], qn[0:64, :])
                                    nc.vector.tensor_mul(qn[:], qn[:], cs_t[:])
                                    nc.vector.tensor_mul(qsw[:], qsw[:], sn_t[:])
                                    if m < HL:
                                        stg = tmpA.tile([128, CHUNK_A], F32R, tag="stg")
                                        nc.vector.tensor_add(stg[:], qn[:], qsw[:])
                                        nc.sync.dma_start(out=qT_dram[m][:, tsl], in_=stg[:])
                                    else:
                                        nc.vector.tensor_add(kT[:, tsl], qn[:], qsw[:])
                                else:
                                    # v: copy + transpose into [s, d] tiles
                                    vstg = tmpA.tile([128, CHUNK_A], F32, tag="vstg")
                                    nc.scalar.activation(vstg[:], psq[:], AF.Copy)
                                    for j in range(CHUNK_A // 128):
                                        vt_ps = psvt_p.tile([128, 128], F32, tag="vt")
                                        nc.tensor.transpose(
                                            vt_ps[:], vstg[:, j * 128:(j + 1) * 128], ident[:]
                                        )
                                        st_i = tci * (CHUNK_A // 128) + j
                                        nc.vector.tensor_copy(vnat[:, st_i, :], vt_ps[:])
                            return _post

                        pending_post[0] = make_post()
                flush_post()

            # ---------------- Phase B: attention + o_proj + reduce-scatter ---
            with (
                tc.tile_pool(name="persistB", bufs=1) as pB,
                tc.tile_pool(name="qvp", bufs=3) as qvp,
                tc.tile_pool(name="exp", bufs=6) as exp_p,
                tc.tile_pool(name="stB", bufs=2) as stB,
                tc.tile_pool(name="ostg", bufs=3) as ostg_p,
                tc.tile_pool(name="cvp", bufs=1) as cvp,
                tc.tile_pool(name="psA", bufs=3, space="PSUM") as psA_p,
                tc.tile_pool(name="psav", bufs=2, space="PSUM") as psav_p,
                tc.tile_pool(name="pssum", bufs=2, space="PSUM") as pssum_p,
                tc.tile_pool(name="psbcB", bufs=1, space="PSUM") as psbcB_p,
                tc.tile_pool(name="dramB", bufs=1, space="DRAM") as dramB,
            ):
                attnT = pB.tile([128, HL, T], F32R)
                wo_sb = pB.tile([128, HL, HID], F32R)
                mask_sb = pB.tile([128, len(MASK_DELTAS), QC], F32)
                nc.gpsimd.dma_start(out=wo_sb[:], in_=wo[:])
                nc.gpsimd.dma_start(out=mask_sb[:], in_=masks[:])

                partial = [
                    dramB.tile([QC, HID], BF16, name=f"partial{qc}") for qc in range(N_QC)
                ]
                rs_out = [
                    dramB.tile([QC // N_CORES, HID], BF16, name=f"rsout{qc}")
                    for qc in range(N_QC)
                ]

                pending_norm = [None]

                def flush_norm():
                    if pending_norm[0] is not None:
                        pending_norm[0]()
                        pending_norm[0] = None

                for qc in range(N_QC):
                    qsl = slice(qc * QC, (qc + 1) * QC)
                    si_lo = max(0, 4 * qc - 8)
                    si_hi = 4 * qc + 3
                    sis = list(range(si_lo, si_hi + 1))
                    for hp in range(0, HL, 2):
                        qvs, avs, sums = [], [], []
                        for h in (hp, hp + 1):
                            qv = qvp.tile([128, QC], F32R, tag="qv")
                            nc.sync.dma_start(out=qv[:], in_=qT_dram[h][:, qsl])
                            qvs.append(qv)
                            avs.append(psav_p.tile([128, QC], F32, tag="av", name="av"))
                            sums.append(pssum_p.tile([1, QC], F32, tag="sum", name="sum"))
                        exs = {}

                        # two heads share kT/vnat tiles; their independent
                        # score->exp->consume chains interleave so ACT/DVE
                        # latency hides under PE work
                        def emit_scores(si):
                            for j in range(2):
                                psc = psA_p.tile([128, QC], F32, tag="sc")
                                nc.tensor.matmul(
                                    psc[:], kT[:, si * 128:(si + 1) * 128],
                                    qvs[j][:], start=True, stop=True,
                                )
                                ex = exp_p.tile([128, QC], F32R, tag="ex")
                                nc.scalar.activation(ex[:], psc[:], AF.Exp)
                                delta = qc * QC - si * 128
                                if delta in MASK_IDX:
                                    nc.vector.tensor_mul(
                                        ex[:], ex[:], mask_sb[:, MASK_IDX[delta], :]
                                    )
                                exs[(si, j)] = ex

                        def emit_consume(si):
                            first = si == sis[0]
                            last = si == sis[-1]
                            for j in range(2):
                                ex = exs.pop((si, j))
                                nc.tensor.matmul(
                                    sums[j][:], onc[:], ex[:], start=first, stop=last
                                )
                                nc.tensor.matmul(
                                    avs[j][:], vnat[:, si, :], ex[:], start=first, stop=last
                                )

                        emit_scores(sis[0])
                        flush_norm()
                        for si in sis[1:]:
                            emit_scores(si)
                            emit_consume(si - 1)
                        emit_consume(sis[-1])

                        def make_norm(hp=hp, avs=avs, sums=sums, qsl=qsl):
                            def _norm():
                                for j in range(2):
                                    rc = stB.tile([1, QC], F32R, tag="rc")
                                    with nc.allow_low_precision(reason="f32r recip"):
                                        nc.vector.reciprocal(rc[:], sums[j][:])
                                    bcp = psbcB_p.tile([128, QC], F32, tag="bcB")
                                    nc.tensor.matmul(bcp[:], onr[:], rc[:], start=True, stop=True)
                                    at = attnT[:, hp + j, qsl]
                                    nc.scalar.activation(at, avs[j][:], AF.Copy)
                                    nc.vector.tensor_mul(at, at, bcp[:])
                            return _norm

                        pending_norm[0] = make_norm()
                    flush_norm()

                    # o_proj for this 512-row slab
                    for tt in range(QC // 128):
                        trow = qc * 4 + tt
                        for ec in range(N_ECH):
                            pso = psA_p.tile([128, ECH], F32, tag="sc")
                            for h in range(HL):
                                nc.tensor.matmul(
                                    pso[:],
                                    attnT[:, h, trow * 128:(trow + 1) * 128],
                                    wo_sb[:, h, ec * ECH:(ec + 1) * ECH],
                                    start=(h == 0),
                                    stop=(h == HL - 1),
                                )
                            ost = ostg_p.tile([128, ECH], BF16, tag="ost")
                            nc.any.tensor_copy(out=ost[:], in_=pso[:])
                            nc.sync.dma_start(
                                out=partial[qc][tt * 128:(tt + 1) * 128,
                                                ec * ECH:(ec + 1) * ECH],
                                in_=ost[:],
                            )
                    nc.gpsimd.collective_compute(
                        "ReduceScatter",
                        mybir.AluOpType.add,
                        replica_groups=[list(range(N_CORES))],
                        ins=[partial[qc][:]],
                        outs=[rs_out[qc][:]],
                    )

                # rs->out conversions at the very end (gpsimd queues) so a
                # DMA waiting on a collective never head-of-line-blocks the
                # compute DMA queues
                for qc in range(N_QC):
                    cv_b = cvp.tile([QC // N_CORES, HID], BF16, tag="cvb")
                    cv_f = cvp.tile([QC // N_CORES, HID], F32, tag="cvf")
                    nc.gpsimd.dma_start(out=cv_b[:], in_=rs_out[qc][:])
                    nc.vector.tensor_copy(cv_f[:], cv_b[:])
                    nc.gpsimd.dma_start(out=out_p[qc], in_=cv_f[:])

    nc.finalize()
    return nc


_NC_CACHE = None


def _get_nc():
    global _NC_CACHE
    if _NC_CACHE is None:
        _NC_CACHE = _build()
    return _NC_CACHE


def _host_inputs(positions, hidden_states, w_qkv, q_norm_w, k_norm_w, w_o):
    positions = np.asarray(positions)
    hidden_states = np.asarray(hidden_states, dtype=np.float32)
    w_qkv = np.asarray(w_qkv, dtype=np.float32)
    q_norm_w = np.asarray(q_norm_w, dtype=np.float32)
    k_norm_w = np.asarray(k_norm_w, dtype=np.float32)
    w_o = np.asarray(w_o, dtype=np.float32)

    # [ki, tc, ko, j]: hidT4[ki, tc, ko, j] = hidden[tc*CHUNK_A+j, ko*128+ki]
    hidT4 = np.ascontiguousarray(
        hidden_states.T.reshape(KO, 128, N_CHUNKS_A, CHUNK_A).transpose(1, 2, 0, 3)
    )

    half = D // 2
    inv_freq = 1.0 / (THETA ** (np.arange(half, dtype=np.float32) / half))
    ang = positions.astype(np.float32)[:, None] * inv_freq[None, :]  # [T, 64]
    cos = np.cos(ang).T.astype(np.float32)   # [64, T]
    sin = np.sin(ang).T.astype(np.float32)
    cs2 = np.concatenate([cos, cos], axis=0)          # [128, T]
    sn2s = np.concatenate([-sin, sin], axis=0)        # [128, T]

    mk = np.zeros((len(MASK_DELTAS), 128, QC), np.float32)
    ss = np.arange(128)[:, None]
    ttv = np.arange(QC)[None, :]
    for i, dlt in enumerate(MASK_DELTAS):
        diff = dlt + ttv - ss
        mk[i] = ((diff >= 0) & (diff < WINDOW)).astype(np.float32)
    mk = np.ascontiguousarray(mk.transpose(1, 0, 2))  # [128, nmask, QC]

    qwv = (q_norm_w * (D ** -0.5)).reshape(D, 1).astype(np.float32)
    kwv = k_norm_w.reshape(D, 1).astype(np.float32)
    onc = np.ones((128, 1), np.float32)
    onr = np.ones((1, 128), np.float32)

    in_maps = []
    for c in range(N_CORES):
        wq_c = np.concatenate(
            [
                w_qkv[:, c * HL * D:(c + 1) * HL * D],
                w_qkv[:, H * D + c * D:H * D + (c + 1) * D],
                w_qkv[:, (H + HKV) * D + c * D:(H + HKV) * D + (c + 1) * D],
            ],
            axis=1,
        )
        # [ki, ko, m]
        wq_c = np.ascontiguousarray(wq_c.reshape(KO, 128, (HL + 2) * D).transpose(1, 0, 2))
        # [p, h, e]
        wo_c = np.ascontiguousarray(
            w_o[c * HL * D:(c + 1) * HL * D, :].reshape(HL, 128, HID).transpose(1, 0, 2)
        )
        in_maps.append(
            {
                "hidT": hidT4,
                "wq": wq_c,
                "wo": wo_c,
                "cs2": cs2,
                "sn2s": sn2s,
                "masks": mk,
                "qwv": qwv,
                "kwv": kwv,
                "onc": onc,
                "onr": onr,
            }
        )
    return in_maps


def _assemble(results):
    out = np.empty((T, HID), np.float32)
    rows = QC // N_CORES
    for c in range(N_CORES):
        r = results[c]["out"]  # [N_QC, rows, HID]
        for qc in range(N_QC):
            out[qc * QC + c * rows: qc * QC + (c + 1) * rows] = r[qc]
    return out


def run_spmd(in_maps, trace=False, **kw):
    nc = _get_nc()
    return run_bass_kernel_spmd(nc, in_maps, list(range(N_CORES)), trace=trace, **kw)


def kernel(positions, hidden_states, w_qkv, q_norm_w, k_norm_w, w_o):
    in_maps = _host_inputs(positions, hidden_states, w_qkv, q_norm_w, k_norm_w, w_o)
    res = run_spmd(in_maps)
    return _assemble(res.results)
